# revision 2
# baseline (speedup 1.0000x reference)
"""AttentionWithRoPE Trainium2 kernel (8-core SPMD).

Sharding: core c handles batch b = c // 2 and head-group g = c % 2
(heads 4g..4g+3).  Host sums the two partial outputs per batch.

Key ideas vs the original baseline:
- All attention matmuls are bf16 AND full 128x128-array ops.  The HAM
  clock gate only counts full-array matmuls as "PE busy": half-array
  ops (K=64 scores / M=65 attn@V) leave the PE throttled at 1.2 GHz
  for the whole attention phase (measured: cold 300us/387us).  Scores
  therefore contract over K=128 with the other head's rows zero-padded
  in the stationary operand, and attn@V pads V's 65 columns (64 dims +
  ones-denominator) to 128.  Zero padding adds no cycles.
- Q/K projection runs on RAW x in f32r (bf16-quantizing x was the
  dominant error term); the rmsnorm scale is folded into the PSUM
  drain (DVE: psum * sinv -> bf16), so projection matmuls never wait
  on the norm.
- rmsnorm: Ln+Exp (one act table set shared with the softmax Exp).
- Softmax exp is split ACT (table exp) / DVE (Schraudolph bf16:
  bits_i16 = round(A*s + B) bitcast to bf16) to share the 16.8M-elem
  exp wall across two engines.
- Output bf16 (host converts + sums the partial pairs).
"""

import os
from contextlib import ExitStack

import numpy as np

import concourse.bass as bass
import concourse.tile as tile
from concourse import bacc, mybir

B, N, DIM = 4, 2048, 512
H, D = 8, 64
ROPE_THETA = 10000.0
NCORES = 8
SCALE = D ** -0.5

F32 = mybir.dt.float32
F32R = mybir.dt.float32r
BF16 = mybir.dt.bfloat16
FP16 = mybir.dt.float16
I16 = mybir.dt.int16

# Fraction (in 8ths) of softmax-exp tiles computed on DVE via the
# Schraudolph bf16 bit trick instead of ACT.
DVE_8 = int(os.environ.get("KERNEL_DVE_8", "2"))
# Schraudolph intercept: 127*128 - C (C tunes the error balance) plus
# +0.5 if the DVE f32->i16 convert truncates instead of rounding.
DVE_B = float(os.environ.get("KERNEL_DVE_B", "16250.5"))
DVE_A = SCALE * 128.0 / np.log(2.0)  # folds the 1/sqrt(d) logit scale

LN_SQRT_DIM = float(0.5 * np.log(DIM))  # bias for sinv = exp(-0.5 ln ss + b)


def use_dve(qh, hp, kt, j):
    """Pick the engine for each softmax-exp tile.

    The first two key-tiles of every block stay on ACT so the normalize
    chain queued on DVE at the block boundary can't stall the new
    block's pipeline.  The rest spread DVE_8/8ths onto DVE.
    """
    if kt < 2:
        return False
    idx = ((qh * 2 + hp) * 16 + kt) * 2 + j
    return (idx * DVE_8) % 8 < DVE_8


def build_program():
    nc = bacc.Bacc("TRN2", target_bir_lowering=False, debug=False)

    xf_d = nc.dram_tensor("xf", [DIM, N], F32R, kind="ExternalInput").ap()
    xb_d = nc.dram_tensor("xb", [DIM, N], BF16, kind="ExternalInput").ap()
    wqk_d = nc.dram_tensor("wqk", [DIM, 512], F32R, kind="ExternalInput").ap()
    wv_d = nc.dram_tensor("wv", [DIM, 256], BF16, kind="ExternalInput").ap()
    wo_d = nc.dram_tensor("wo", [256, DIM], BF16, kind="ExternalInput").ap()
    cos_d = nc.dram_tensor("cos2", [128, N], FP16, kind="ExternalInput").ap()
    sinF_d = nc.dram_tensor("sinF2", [128, N], FP16, kind="ExternalInput").ap()
    yT = nc.dram_tensor("yT", [DIM, N], BF16, kind="ExternalOutput").ap()

    with tile.TileContext(nc) as tc:
        with ExitStack() as ctx:
            persist = ctx.enter_context(tc.tile_pool(name="persist", bufs=1))
            xsqp = ctx.enter_context(tc.tile_pool(name="xsqp", bufs=2))
            ropew = ctx.enter_context(tc.tile_pool(name="ropew", bufs=2))
            rotup = ctx.enter_context(tc.tile_pool(name="rotup", bufs=2))
            cosp = ctx.enter_context(tc.tile_pool(name="cosp", bufs=2))
            ps_s = ctx.enter_context(tc.tile_pool(name="ps_s", bufs=2, space="PSUM"))
            ps_o = ctx.enter_context(tc.tile_pool(name="ps_o", bufs=2, space="PSUM"))
            exps = ctx.enter_context(tc.tile_pool(name="exps", bufs=4))
            rcp = ctx.enter_context(tc.tile_pool(name="rcp", bufs=2))
            ysb = ctx.enter_context(tc.tile_pool(name="ysb", bufs=1))

            # ---- input DMAs ----
            xb = []
            for i in range(4):
                t = persist.tile([128, N], BF16, tag=f"xb{i}", name=f"xb{i}")
                nc.sync.dma_start(t[:], xb_d[i * 128:(i + 1) * 128, :])
                xb.append(t)
            wqk_t = []
            for i in range(4):
                t = persist.tile([128, 512], F32R, tag=f"wqk{i}", name=f"wqk{i}")
                nc.sync.dma_start(t[:], wqk_d[i * 128:(i + 1) * 128, :])
                wqk_t.append(t)
            xf = []
            for i in range(4):
                t = persist.tile([128, N], F32R, tag=f"xf{i}", name=f"xf{i}")
                nc.sync.dma_start(t[:], xf_d[i * 128:(i + 1) * 128, :])
                xf.append(t)
            cos_t = persist.tile([128, N], FP16, tag="cos", name="cos")
            nc.sync.dma_start(cos_t[:], cos_d)
            sinF_t = persist.tile([128, N], FP16, tag="sinF", name="sinF")
            nc.sync.dma_start(sinF_t[:], sinF_d)
            wv_t = []
            for i in range(4):
                t = persist.tile([128, 256], BF16, tag=f"wv{i}", name=f"wv{i}")
                nc.sync.dma_start(t[:], wv_d[i * 128:(i + 1) * 128, :])
                wv_t.append(t)
            wo_t = []
            for p in range(2):
                t = persist.tile([128, 512], BF16, tag=f"wo{p}", name=f"wo{p}")
                nc.sync.dma_start(t[:], wo_d[p * 128:(p + 1) * 128, :])
                wo_t.append(t)

            ones128 = persist.tile([128, 128], BF16, tag="ones", name="ones")
            nc.vector.memset(ones128[:], 1.0)

            # Warm-up batches: full-array matmuls that fill the PE while the
            # input DMAs land / phase A waits on DVE, so HAM reaches (and
            # keeps) K=8/8 before the projection phase.
            def emit_warm(n):
                warm = ps_s.tile([128, 512], F32, tag="sc", name="warm")
                for w in range(n):
                    nc.tensor.matmul(warm[:, 0:128], ones128[:], ones128[:],
                                     start=True, stop=True)

            emit_warm(24)

            # kz[h]: zero-padded per-head roped K. Rows 0:64 = head h's 64
            # dims, rows 64:128 stay zero so the K=128 scores contraction
            # ignores the other head stacked in qr.
            # Even heads: k on rows 0:64 (zeros below); odd heads: k on rows
            # 64:128 (zeros above) — matching where that head's q lives in qr.
            kz = []
            for h in range(4):
                t = persist.tile([128, N], BF16, tag=f"kz{h}", name=f"kz{h}")
                if h % 2 == 0:
                    nc.gpsimd.memset(t[64:128, :], 0.0)
                else:
                    nc.gpsimd.memset(t[0:64, :], 0.0)
                kz.append(t)

            # v_big: 16 key-tiles x 4 heads x [64 v-dims | ones | 63 zeros]
            # (padded to 128 so attn@V is a full-array matmul).
            v_big = persist.tile([128, 16 * 512], BF16, tag="vbig", name="vbig")
            vb4 = v_big[:].rearrange("p (t h c) -> p t h c", t=16, h=4)
            nc.gpsimd.memset(v_big[:], 0.0)
            nc.vector.tensor_copy(
                vb4[:, :, :, 64],
                ones128[:, 0:64].rearrange("p (t h) -> p t h", t=16))

            # ---- phase A: rmsnorm scale (projections don't wait on it) ----
            lns = persist.tile([128, N], F32, tag="lns", name="lns")
            sinv_b = persist.tile([128, N], BF16, tag="sinvb", name="sinvb")
            bias_t = persist.tile([128, 1], F32, tag="bias", name="bias")
            nc.vector.memset(bias_t[:], LN_SQRT_DIM)

            # All Ln's land in sinv_f, then two Exp passes (bf16 copy, then
            # in-place) — grouping by table set avoids ACT table thrash.
            for c in range(4):
                cs = slice(c * 512, (c + 1) * 512)
                ss = ps_s.tile([128, 512], F32, tag="sc", name="ss")
                for i in range(4):
                    xsq = xsqp.tile([128, 512], BF16, tag="xsq", name="xsq")
                    nc.vector.tensor_mul(xsq[:], xb[i][:, cs], xb[i][:, cs])
                    nc.tensor.matmul(ss[:], ones128[:], xsq[:],
                                     start=(i == 0), stop=(i == 3))
                nc.scalar.activation(lns[:, cs], ss[:],
                                     mybir.ActivationFunctionType.Ln)
                emit_warm(8)
            # sinv = sqrt(DIM)*ss^-0.5 = exp(-0.5*ln(ss) + ln(sqrt(DIM)))
            nc.scalar.activation(sinv_b[:], lns[:],
                                 mybir.ActivationFunctionType.Exp,
                                 bias=bias_t[:], scale=-0.5)
            nc.scalar.activation(lns[:], lns[:],
                                 mybir.ActivationFunctionType.Exp,
                                 bias=bias_t[:], scale=-0.5)
            # Q/K projections run on RAW x; the rmsnorm scale rides in the
            # rope tables instead: rope(x*sinv) = raw*(cos*sinv) +
            # swap(raw)*(sinF*sinv) — sinv is per-token, rope mixes dims.
            # The whole rope intermediate path is fp16 (11-bit mantissa);
            # only the final add rounds to bf16, so the per-token scale and
            # the rotation see a single coarse rounding.
            cosS = persist.tile([128, N], FP16, tag="cosS", name="cosS")
            nc.vector.tensor_mul(cosS[:], cos_t[:], lns[:])
            sinFS = persist.tile([128, N], FP16, tag="sinFS", name="sinFS")
            nc.vector.tensor_mul(sinFS[:], sinF_t[:], lns[:])
            # xnb (bf16, for v proj), in place
            for i in range(4):
                nc.vector.tensor_mul(xb[i][:], xb[i][:], sinv_b[:])

            # ---- Q/K projection (raw x) + folded-norm RoPE ----
            # m: 0 = q heads(0,1), 1 = q heads(2,3), 2 = k(0,1), 3 = k(2,3)
            qk_dest = [persist.tile([128, N], BF16, tag=nm, name=nm)
                       for nm in ["qr0", "qr1"]]

            def emit_rope(m):
                ms = slice(m * 128, (m + 1) * 128)
                raw = ropew.tile([128, N], FP16, tag="raw", name=f"raw{m}")
                for c in range(4):
                    cs = slice(c * 512, (c + 1) * 512)
                    qk = ps_s.tile([128, 512], F32, tag="sc", name="qkps")
                    for i in range(4):
                        nc.tensor.matmul(qk[:], wqk_t[i][:, ms], xf[i][:, cs],
                                         start=(i == 0), stop=(i == 3))
                    nc.scalar.activation(raw[:, cs], qk[:],
                                         mybir.ActivationFunctionType.Copy)
                # pair-swap via SBUF->SBUF DMA (TENSOR_TENSOR needs equal
                # start partitions for SBUF operands; DMA moves partitions
                # freely), then aligned muls.
                rotu = rotup.tile([128, N], FP16, tag="rotu", name=f"rotu{m}")
                for h0 in (0, 64):
                    nc.sync.dma_start(rotu[h0:h0 + 32, :],
                                      raw[h0 + 32:h0 + 64, :])
                    nc.sync.dma_start(rotu[h0 + 32:h0 + 64, :],
                                      raw[h0:h0 + 32, :])
                nc.vector.tensor_mul(rotu[:], rotu[:], sinFS[:])
                qkcos = cosp.tile([128, N], FP16, tag="qkcos", name=f"qkcos{m}")
                nc.vector.tensor_mul(qkcos[:], raw[:], cosS[:])
                if m < 2:
                    nc.vector.tensor_add(qk_dest[m][:], qkcos[:], rotu[:])
                else:
                    # K: write each head's rows straight into its
                    # zero-padded kz tile (partition-aligned adds).
                    hp = m - 2
                    nc.vector.tensor_add(kz[2 * hp][0:64, :],
                                         qkcos[0:64, :], rotu[0:64, :])
                    nc.vector.tensor_add(kz[2 * hp + 1][64:128, :],
                                         qkcos[64:128, :], rotu[64:128, :])

            def emit_v():
                for tt in range(16):
                    ts = slice(tt * 128, (tt + 1) * 128)
                    vp = ps_s.tile([128, 512], F32, tag="sc", name="vps")
                    for i in range(4):
                        nc.tensor.matmul(vp[:, 0:256], xb[i][:, ts], wv_t[i][:],
                                         start=(i == 0), stop=(i == 3))
                    nc.scalar.activation(
                        vb4[:, tt, :, 0:64],
                        vp[:, 0:256].rearrange("p (h c) -> p h c", h=4),
                        mybir.ActivationFunctionType.Copy)

            # ---- attention ----
            outd = [[None, None], [None, None]]

            def emit_attention(qh, hp):
                qr = qk_dest[hp]
                o_ps = [ps_o.tile([128, 1024], F32, tag="o", name="o")
                        for _ in range(2)]
                def emit_av(kt, e_t):
                    for j in range(2):
                        h = 2 * hp + j
                        vcol = slice(512 * kt + 128 * h,
                                     512 * kt + 128 * h + 128)
                        for sub in range(2):
                            ss_ = slice(sub * 512, (sub + 1) * 512)
                            nc.tensor.matmul(
                                o_ps[j][:, ss_],
                                v_big[:, vcol], e_t[j][:, ss_],
                                start=(kt == 0), stop=(kt == 15),
                                skip_group_check=True)

                # software pipeline: attn@V for key-tile kt-1 is emitted
                # after scores(kt), so the PE has work while exp(kt) runs.
                prev = None
                for kt in range(16):
                    ks = slice(kt * 128, (kt + 1) * 128)
                    e_t = []
                    for j in range(2):
                        s_ps = ps_s.tile([128, 1024], F32, tag="sc", name="sc")
                        for sub in range(2):
                            qs = slice(qh * 1024 + sub * 512,
                                       qh * 1024 + (sub + 1) * 512)
                            nc.tensor.matmul(
                                s_ps[:, sub * 512:(sub + 1) * 512],
                                kz[2 * hp + j][:, ks], qr[:, qs],
                                start=True, stop=True)
                        e = exps.tile([128, 1024], BF16, tag="e", name="e")
                        if use_dve(qh, hp, kt, j):
                            nc.vector.tensor_scalar(
                                e[:].bitcast(I16), s_ps[:],
                                float(DVE_A), float(DVE_B),
                                mybir.AluOpType.mult, mybir.AluOpType.add)
                        else:
                            nc.scalar.activation(
                                e[:], s_ps[:],
                                mybir.ActivationFunctionType.Exp, scale=SCALE)
                        e_t.append(e)
                    if prev is not None:
                        emit_av(kt - 1, prev)
                    prev = e_t
                emit_av(15, prev)
                od = persist.tile([128, 1024], BF16, tag=f"od{hp}{qh}",
                                  name=f"od{hp}{qh}")
                outd[hp][qh] = od
                for j in range(2):
                    js = slice(j * 64, (j + 1) * 64)
                    rrow = rcp.tile([1, 1024], F32, tag="rrow", name="rrow")
                    dcopy = rcp.tile([1, 1024], F32, tag="dcopy", name="dcopy")
                    nc.vector.tensor_copy(dcopy[:], o_ps[j][64:65, :])
                    nc.vector.reciprocal_approx_fast(rrow[:], dcopy[:])
                    rfull = rcp.tile([64, 1024], F32, tag="rfull", name="rfull")
                    nc.gpsimd.partition_broadcast(rfull[:], rrow[:])
                    nc.vector.tensor_mul(od[js, :], o_ps[j][0:64, :], rfull[:])

            def emit_proj(qh):
                for om in range(4):
                    oms = slice(om * 128, (om + 1) * 128)
                    yp = ps_s.tile([128, 1024], F32, tag="sc", name="yp")
                    for sub in range(2):
                        ss_ = slice(sub * 512, (sub + 1) * 512)
                        for p in range(2):
                            nc.tensor.matmul(
                                yp[:, ss_], wo_t[p][:, oms],
                                outd[p][qh][:, ss_],
                                start=(p == 0), stop=(p == 1))
                    yo = ysb.tile([128, 1024], BF16, tag="y", name="y")
                    if om % 2 == 0:
                        nc.scalar.activation(yo[:], yp[:],
                                             mybir.ActivationFunctionType.Copy)
                    else:
                        nc.vector.tensor_copy(yo[:], yp[:])
                    nc.sync.dma_start(
                        yT[oms, qh * 1024:(qh + 1) * 1024], yo[:])

            # ---- emission order: the (0,0) block's K and Q first (they
            # only need raw x), V before attention, second pair last ----
            emit_rope(2)
            emit_rope(0)
            emit_v()
            emit_rope(3)
            emit_rope(1)
            emit_attention(0, 0)
            emit_attention(0, 1)
            emit_attention(1, 0)
            emit_proj(0)
            emit_attention(1, 1)
            emit_proj(1)

    nc.compile()
    return nc


def rope_tables():
    """cos / sign-folded sin tables, [evens | odds] row order, two 64-row
    head slots stacked twice to fill 128 partitions."""
    inv_freq = (1.0 / (ROPE_THETA ** (np.arange(0, D, 2, dtype=np.float32) / D)))
    freqs = np.arange(N, dtype=np.float32)[:, None] * inv_freq[None, :]  # [N, 32]
    cos = np.cos(freqs).T.astype(np.float32)  # [32, N]
    sin = np.sin(freqs).T.astype(np.float32)
    cos64 = np.concatenate([cos, cos], axis=0)
    sinF64 = np.concatenate([-sin, sin], axis=0)
    cos2 = np.concatenate([cos64, cos64], axis=0)  # [128, N]
    sinF2 = np.concatenate([sinF64, sinF64], axis=0)
    return np.ascontiguousarray(cos2), np.ascontiguousarray(sinF2)


_PERM64 = np.concatenate([np.arange(0, D, 2), np.arange(1, D, 2)])


def _permute_heads(w):
    """Permute each head's 64 columns of w [512, 256] to [evens | odds]."""
    w = w.reshape(DIM, 4, D)[:, :, _PERM64]
    return w.reshape(DIM, 256)


def _bf16():
    import ml_dtypes
    return ml_dtypes.bfloat16


def make_in_maps(x, gamma, w_qkv, w_out):
    bf = _bf16()
    cos2, sinF2 = rope_tables()
    cos2b = cos2.astype(np.float16)
    sinF2b = sinF2.astype(np.float16)
    wg = (gamma[:, None] * w_qkv).astype(np.float32)  # fold gamma
    in_maps = []
    for c in range(NCORES):
        b, g = c // 2, c % 2
        hs = slice(g * 256, (g + 1) * 256)
        wqk_c = np.concatenate([_permute_heads(wg[:, 0:512][:, hs]),
                                _permute_heads(wg[:, 512:1024][:, hs])],
                               axis=1)
        wv_c = wg[:, 1024:1536][:, hs]
        wo_c = w_out[hs, :]
        xT = np.ascontiguousarray(x[b].T).astype(np.float32)
        in_maps.append({
            "xf": xT,
            "xb": xT.astype(bf),
            "wqk": np.ascontiguousarray(wqk_c),
            "wv": np.ascontiguousarray(wv_c).astype(bf),
            "wo": np.ascontiguousarray(wo_c).astype(bf),
            "cos2": cos2b,
            "sinF2": sinF2b,
        })
    return in_maps


_NC_CACHE = None


def _get_program():
    global _NC_CACHE
    if _NC_CACHE is None:
        _NC_CACHE = build_program()
    return _NC_CACHE


def run_cores(inputs, trace=False):
    """Run the SPMD kernel on 8 cores; returns (full_output, results)."""
    from concourse.bass_utils import run_bass_kernel_spmd

    nc = _get_program()
    in_maps = make_in_maps(inputs["x"], inputs["gamma"],
                           inputs["w_qkv"], inputs["w_out"])
    kwargs = {}
    if trace:
        _install_ntff_hook()
        kwargs = dict(trace=True, trace_cores=list(range(NCORES)))
    res = run_bass_kernel_spmd(nc, in_maps, core_ids=list(range(NCORES)),
                               **kwargs)
    out = np.empty((B, N, DIM), dtype=np.float32)
    for b in range(B):
        yTv = (res.results[2 * b]["yT"].astype(np.float32)
               + res.results[2 * b + 1]["yT"].astype(np.float32))
        out[b] = yTv.T
    return out, res


def _install_ntff_hook():
    """Register the axon NTFF profiling hook (missing antenv.axon_hooks)."""
    import sys
    import types

    if "antenv.axon_hooks" in sys.modules:
        return
    try:
        import trn_agent_boot.trn_boot as tb
        import concourse.bass_utils as bu

        mod = types.ModuleType("antenv.axon_hooks")
        hook = tb._ntff_profile_via_ctypes("/opt/axon/libaxon_pjrt.so")
        mod.get_axon_ntff_profile_hook = lambda: hook
        sys.modules["antenv.axon_hooks"] = mod
        bu.upload_artifacts = lambda tmpdir: "local://" + tmpdir
    except Exception:
        pass


def kernel(**inputs):
    out, _ = run_cores(inputs, trace=bool(os.environ.get("KERNEL_TRACE")))
    return out


# revision 4
# speedup vs baseline: 1.0301x; 1.0301x over previous
"""AttentionWithRoPE Trainium2 kernel (8-core SPMD).

Sharding: core c handles batch b = c // 2 and head-group g = c % 2
(heads 4g..4g+3).  Host sums the two partial outputs per batch.

Key ideas vs the original baseline:
- All attention matmuls are bf16 AND full 128x128-array ops.  The HAM
  clock gate only counts full-array matmuls as "PE busy": half-array
  ops (K=64 scores / M=65 attn@V) leave the PE throttled at 1.2 GHz
  for the whole attention phase (measured: cold 300us/387us).  Scores
  therefore contract over K=128 with the other head's rows zero-padded
  in the stationary operand, and attn@V pads V's 65 columns (64 dims +
  ones-denominator) to 128.  Zero padding adds no cycles.
- Q/K projection runs on RAW x in f32r (bf16-quantizing x was the
  dominant error term); the rmsnorm scale is folded into the PSUM
  drain (DVE: psum * sinv -> bf16), so projection matmuls never wait
  on the norm.
- rmsnorm: Ln+Exp (one act table set shared with the softmax Exp).
- Softmax exp is split ACT (table exp) / DVE (Schraudolph bf16:
  bits_i16 = round(A*s + B) bitcast to bf16) to share the 16.8M-elem
  exp wall across two engines.
- Output bf16 (host converts + sums the partial pairs).
"""

import os
from contextlib import ExitStack

import numpy as np

import concourse.bass as bass
import concourse.tile as tile
from concourse import bacc, mybir

B, N, DIM = 4, 2048, 512
H, D = 8, 64
ROPE_THETA = 10000.0
NCORES = 8
SCALE = D ** -0.5

F32 = mybir.dt.float32
F32R = mybir.dt.float32r
BF16 = mybir.dt.bfloat16
FP16 = mybir.dt.float16
I16 = mybir.dt.int16

# Fraction (in 8ths) of softmax-exp tiles computed on DVE via the
# Schraudolph bf16 bit trick instead of ACT.
DVE_8 = int(os.environ.get("KERNEL_DVE_8", "2"))
# Schraudolph intercept: 127*128 - C (C tunes the error balance) plus
# +0.5 if the DVE f32->i16 convert truncates instead of rounding.
DVE_B = float(os.environ.get("KERNEL_DVE_B", "16250.5"))
DVE_A = SCALE * 128.0 / np.log(2.0)  # folds the 1/sqrt(d) logit scale

LN_SQRT_DIM = float(0.5 * np.log(DIM))  # bias for sinv = exp(-0.5 ln ss + b)


def use_dve(qh, hp, kt, j):
    """Pick the engine for each softmax-exp tile.

    The first two key-tiles of every block stay on ACT so the normalize
    chain queued on DVE at the block boundary can't stall the new
    block's pipeline.  The rest spread DVE_8/8ths onto DVE.
    """
    if kt < 2:
        return False
    idx = ((qh * 2 + hp) * 16 + kt) * 2 + j
    return (idx * DVE_8) % 8 < DVE_8


def build_program():
    nc = bacc.Bacc("TRN2", target_bir_lowering=False, debug=False)

    xf_d = nc.dram_tensor("xf", [DIM, N], F32R, kind="ExternalInput").ap()
    xb_d = nc.dram_tensor("xb", [DIM, N], BF16, kind="ExternalInput").ap()
    wqk_d = nc.dram_tensor("wqk", [DIM, 512], F32R, kind="ExternalInput").ap()
    wv_d = nc.dram_tensor("wv", [DIM, 256], BF16, kind="ExternalInput").ap()
    wo_d = nc.dram_tensor("wo", [256, DIM], BF16, kind="ExternalInput").ap()
    cos_d = nc.dram_tensor("cos2", [128, N], FP16, kind="ExternalInput").ap()
    sinF_d = nc.dram_tensor("sinF2", [128, N], FP16, kind="ExternalInput").ap()
    yT = nc.dram_tensor("yT", [DIM, N], BF16, kind="ExternalOutput").ap()

    with tile.TileContext(nc) as tc:
        with ExitStack() as ctx:
            persist = ctx.enter_context(tc.tile_pool(name="persist", bufs=1))
            xsqp = ctx.enter_context(tc.tile_pool(name="xsqp", bufs=2))
            ropew = ctx.enter_context(tc.tile_pool(name="ropew", bufs=2))
            rotup = ctx.enter_context(tc.tile_pool(name="rotup", bufs=2))
            cosp = ctx.enter_context(tc.tile_pool(name="cosp", bufs=2))
            ps_s = ctx.enter_context(tc.tile_pool(name="ps_s", bufs=2, space="PSUM"))
            ps_o = ctx.enter_context(tc.tile_pool(name="ps_o", bufs=2, space="PSUM"))
            exps = ctx.enter_context(tc.tile_pool(name="exps", bufs=4))
            rcp = ctx.enter_context(tc.tile_pool(name="rcp", bufs=2))
            ysb = ctx.enter_context(tc.tile_pool(name="ysb", bufs=1))

            # ---- input DMAs ----
            xb = []
            for i in range(4):
                t = persist.tile([128, N], BF16, tag=f"xb{i}", name=f"xb{i}")
                nc.sync.dma_start(t[:], xb_d[i * 128:(i + 1) * 128, :])
                xb.append(t)
            wqk_t = []
            for i in range(4):
                t = persist.tile([128, 512], F32R, tag=f"wqk{i}", name=f"wqk{i}")
                nc.sync.dma_start(t[:], wqk_d[i * 128:(i + 1) * 128, :])
                wqk_t.append(t)
            xf = []
            for i in range(4):
                t = persist.tile([128, N], F32R, tag=f"xf{i}", name=f"xf{i}")
                nc.sync.dma_start(t[:], xf_d[i * 128:(i + 1) * 128, :])
                xf.append(t)
            cos_t = persist.tile([128, N], FP16, tag="cos", name="cos")
            nc.sync.dma_start(cos_t[:], cos_d)
            sinF_t = persist.tile([128, N], FP16, tag="sinF", name="sinF")
            nc.sync.dma_start(sinF_t[:], sinF_d)
            wv_t = []
            for i in range(4):
                t = persist.tile([128, 256], BF16, tag=f"wv{i}", name=f"wv{i}")
                nc.sync.dma_start(t[:], wv_d[i * 128:(i + 1) * 128, :])
                wv_t.append(t)
            wo_t = []
            for p in range(2):
                t = persist.tile([128, 512], BF16, tag=f"wo{p}", name=f"wo{p}")
                nc.sync.dma_start(t[:], wo_d[p * 128:(p + 1) * 128, :])
                wo_t.append(t)

            ones128 = persist.tile([128, 128], BF16, tag="ones", name="ones")
            nc.vector.memset(ones128[:], 1.0)

            # Warm-up batches: full-array matmuls that fill the PE while the
            # input DMAs land / phase A waits on DVE, so HAM reaches (and
            # keeps) K=8/8 before the projection phase.  They draw from the
            # ps_o pool (idle until attention) so they never contend with
            # the rmsnorm chunks for ps_s slots.
            def emit_warm(n):
                warm = ps_o.tile([128, 512], F32, tag="o", name="warm")
                for w in range(n):
                    nc.tensor.matmul(warm[:, 0:128], ones128[:], ones128[:],
                                     start=True, stop=True)

            emit_warm(40)

            # kz[h]: zero-padded per-head roped K. Rows 0:64 = head h's 64
            # dims, rows 64:128 stay zero so the K=128 scores contraction
            # ignores the other head stacked in qr.
            # Even heads: k on rows 0:64 (zeros below); odd heads: k on rows
            # 64:128 (zeros above) — matching where that head's q lives in qr.
            kz = []
            for h in range(4):
                t = persist.tile([128, N], BF16, tag=f"kz{h}", name=f"kz{h}")
                if h % 2 == 0:
                    nc.gpsimd.memset(t[64:128, :], 0.0)
                else:
                    nc.gpsimd.memset(t[0:64, :], 0.0)
                kz.append(t)

            # v_big: 16 key-tiles x 4 heads x [64 v-dims | ones | 63 zeros]
            # (padded to 128 so attn@V is a full-array matmul).
            v_big = persist.tile([128, 16 * 512], BF16, tag="vbig", name="vbig")
            vb4 = v_big[:].rearrange("p (t h c) -> p t h c", t=16, h=4)
            nc.gpsimd.memset(v_big[:], 0.0)
            nc.vector.tensor_copy(
                vb4[:, :, :, 64],
                ones128[:, 0:64].rearrange("p (t h) -> p t h", t=16))

            # ---- phase A: rmsnorm scale (projections don't wait on it) ----
            lns = persist.tile([128, N], F32, tag="lns", name="lns")
            sinv_b = persist.tile([128, N], BF16, tag="sinvb", name="sinvb")
            bias_t = persist.tile([128, 1], F32, tag="bias", name="bias")
            nc.vector.memset(bias_t[:], LN_SQRT_DIM)

            # All Ln's land in sinv_f, then two Exp passes (bf16 copy, then
            # in-place) — grouping by table set avoids ACT table thrash.
            for c in range(4):
                cs = slice(c * 512, (c + 1) * 512)
                ss = ps_s.tile([128, 512], F32, tag="sc", name="ss")
                for i in range(4):
                    xsq = xsqp.tile([128, 512], BF16, tag="xsq", name="xsq")
                    nc.vector.tensor_mul(xsq[:], xb[i][:, cs], xb[i][:, cs])
                    nc.tensor.matmul(ss[:], ones128[:], xsq[:],
                                     start=(i == 0), stop=(i == 3))
                nc.scalar.activation(lns[:, cs], ss[:],
                                     mybir.ActivationFunctionType.Ln)
                emit_warm(8)
            # sinv = sqrt(DIM)*ss^-0.5 = exp(-0.5*ln(ss) + ln(sqrt(DIM)))
            nc.scalar.activation(sinv_b[:], lns[:],
                                 mybir.ActivationFunctionType.Exp,
                                 bias=bias_t[:], scale=-0.5)
            nc.scalar.activation(lns[:], lns[:],
                                 mybir.ActivationFunctionType.Exp,
                                 bias=bias_t[:], scale=-0.5)
            # Q/K projections run on RAW x; the rmsnorm scale rides in the
            # rope tables instead: rope(x*sinv) = raw*(cos*sinv) +
            # swap(raw)*(sinF*sinv) — sinv is per-token, rope mixes dims.
            # The whole rope intermediate path is fp16 (11-bit mantissa);
            # only the final add rounds to bf16, so the per-token scale and
            # the rotation see a single coarse rounding.
            cosS = persist.tile([128, N], FP16, tag="cosS", name="cosS")
            nc.vector.tensor_mul(cosS[:], cos_t[:], lns[:])
            sinFS = persist.tile([128, N], FP16, tag="sinFS", name="sinFS")
            nc.vector.tensor_mul(sinFS[:], sinF_t[:], lns[:])
            # xnb (bf16, for v proj), in place
            for i in range(4):
                nc.vector.tensor_mul(xb[i][:], xb[i][:], sinv_b[:])

            # ---- Q/K projection (raw x) + folded-norm RoPE ----
            # m: 0 = q heads(0,1), 1 = q heads(2,3), 2 = k(0,1), 3 = k(2,3)
            qk_dest = [persist.tile([128, N], BF16, tag=nm, name=nm)
                       for nm in ["qr0", "qr1"]]

            def emit_rope(m):
                ms = slice(m * 128, (m + 1) * 128)
                raw = ropew.tile([128, N], FP16, tag="raw", name=f"raw{m}")
                for c in range(4):
                    cs = slice(c * 512, (c + 1) * 512)
                    qk = ps_s.tile([128, 512], F32, tag="sc", name="qkps")
                    for i in range(4):
                        nc.tensor.matmul(qk[:], wqk_t[i][:, ms], xf[i][:, cs],
                                         start=(i == 0), stop=(i == 3))
                    nc.scalar.activation(raw[:, cs], qk[:],
                                         mybir.ActivationFunctionType.Copy)
                # pair-swap via SBUF->SBUF DMA (TENSOR_TENSOR needs equal
                # start partitions for SBUF operands; DMA moves partitions
                # freely), then aligned muls.
                rotu = rotup.tile([128, N], FP16, tag="rotu", name=f"rotu{m}")
                for h0 in (0, 64):
                    nc.sync.dma_start(rotu[h0:h0 + 32, :],
                                      raw[h0 + 32:h0 + 64, :])
                    nc.sync.dma_start(rotu[h0 + 32:h0 + 64, :],
                                      raw[h0:h0 + 32, :])
                nc.vector.tensor_mul(rotu[:], rotu[:], sinFS[:])
                qkcos = cosp.tile([128, N], FP16, tag="qkcos", name=f"qkcos{m}")
                nc.vector.tensor_mul(qkcos[:], raw[:], cosS[:])
                if m < 2:
                    nc.vector.tensor_add(qk_dest[m][:], qkcos[:], rotu[:])
                else:
                    # K: write each head's rows straight into its
                    # zero-padded kz tile (partition-aligned adds).
                    hp = m - 2
                    nc.vector.tensor_add(kz[2 * hp][0:64, :],
                                         qkcos[0:64, :], rotu[0:64, :])
                    nc.vector.tensor_add(kz[2 * hp + 1][64:128, :],
                                         qkcos[64:128, :], rotu[64:128, :])

            def emit_v():
                for tt in range(16):
                    ts = slice(tt * 128, (tt + 1) * 128)
                    vp = ps_s.tile([128, 512], F32, tag="sc", name="vps")
                    for i in range(4):
                        nc.tensor.matmul(vp[:, 0:256], xb[i][:, ts], wv_t[i][:],
                                         start=(i == 0), stop=(i == 3))
                    nc.scalar.activation(
                        vb4[:, tt, :, 0:64],
                        vp[:, 0:256].rearrange("p (h c) -> p h c", h=4),
                        mybir.ActivationFunctionType.Copy)

            # ---- attention ----
            outd = [[None, None], [None, None]]

            def emit_attention(qh, hp):
                qr = qk_dest[hp]
                o_ps = [ps_o.tile([128, 1024], F32, tag="o", name="o")
                        for _ in range(2)]
                def emit_av(kt, e_t):
                    for j in range(2):
                        h = 2 * hp + j
                        vcol = slice(512 * kt + 128 * h,
                                     512 * kt + 128 * h + 128)
                        for sub in range(2):
                            ss_ = slice(sub * 512, (sub + 1) * 512)
                            nc.tensor.matmul(
                                o_ps[j][:, ss_],
                                v_big[:, vcol], e_t[j][:, ss_],
                                start=(kt == 0), stop=(kt == 15),
                                skip_group_check=True)

                # software pipeline: attn@V for key-tile kt-1 is emitted
                # after scores(kt), so the PE has work while exp(kt) runs.
                prev = None
                for kt in range(16):
                    ks = slice(kt * 128, (kt + 1) * 128)
                    e_t = []
                    for j in range(2):
                        s_ps = ps_s.tile([128, 1024], F32, tag="sc", name="sc")
                        for sub in range(2):
                            qs = slice(qh * 1024 + sub * 512,
                                       qh * 1024 + (sub + 1) * 512)
                            nc.tensor.matmul(
                                s_ps[:, sub * 512:(sub + 1) * 512],
                                kz[2 * hp + j][:, ks], qr[:, qs],
                                start=True, stop=True)
                        e = exps.tile([128, 1024], BF16, tag="e", name="e")
                        if use_dve(qh, hp, kt, j):
                            nc.vector.tensor_scalar(
                                e[:].bitcast(I16), s_ps[:],
                                float(DVE_A), float(DVE_B),
                                mybir.AluOpType.mult, mybir.AluOpType.add)
                        else:
                            nc.scalar.activation(
                                e[:], s_ps[:],
                                mybir.ActivationFunctionType.Exp, scale=SCALE)
                        e_t.append(e)
                    if prev is not None:
                        emit_av(kt - 1, prev)
                    prev = e_t
                emit_av(15, prev)
                od = persist.tile([128, 1024], BF16, tag=f"od{hp}{qh}",
                                  name=f"od{hp}{qh}")
                outd[hp][qh] = od
                for j in range(2):
                    js = slice(j * 64, (j + 1) * 64)
                    rrow = rcp.tile([1, 1024], F32, tag="rrow", name="rrow")
                    dcopy = rcp.tile([1, 1024], F32, tag="dcopy", name="dcopy")
                    nc.vector.tensor_copy(dcopy[:], o_ps[j][64:65, :])
                    nc.vector.reciprocal_approx_fast(rrow[:], dcopy[:])
                    rfull = rcp.tile([64, 1024], F32, tag="rfull", name="rfull")
                    nc.gpsimd.partition_broadcast(rfull[:], rrow[:])
                    nc.vector.tensor_mul(od[js, :], o_ps[j][0:64, :], rfull[:])

            def emit_proj(qh):
                for om in range(4):
                    oms = slice(om * 128, (om + 1) * 128)
                    yp = ps_s.tile([128, 1024], F32, tag="sc", name="yp")
                    for sub in range(2):
                        ss_ = slice(sub * 512, (sub + 1) * 512)
                        for p in range(2):
                            nc.tensor.matmul(
                                yp[:, ss_], wo_t[p][:, oms],
                                outd[p][qh][:, ss_],
                                start=(p == 0), stop=(p == 1))
                    yo = ysb.tile([128, 1024], BF16, tag="y", name="y")
                    if om % 2 == 0:
                        nc.scalar.activation(yo[:], yp[:],
                                             mybir.ActivationFunctionType.Copy)
                    else:
                        nc.vector.tensor_copy(yo[:], yp[:])
                    nc.sync.dma_start(
                        yT[oms, qh * 1024:(qh + 1) * 1024], yo[:])

            # ---- emission order: the (0,0) block's K and Q first (they
            # only need raw x), V before attention, second pair last ----
            emit_rope(2)
            emit_rope(0)
            emit_v()
            emit_rope(3)
            emit_rope(1)
            # proj(0) goes after att(1,1): its matmuls run (and keep HAM
            # warm) while norm(1,1) drains, instead of stalling the PE
            # twice waiting for each qh's last normalize.
            emit_attention(0, 0)
            emit_attention(0, 1)
            emit_attention(1, 0)
            emit_attention(1, 1)
            emit_proj(0)
            emit_proj(1)

    nc.compile()
    return nc


def rope_tables():
    """cos / sign-folded sin tables, [evens | odds] row order, two 64-row
    head slots stacked twice to fill 128 partitions."""
    inv_freq = (1.0 / (ROPE_THETA ** (np.arange(0, D, 2, dtype=np.float32) / D)))
    freqs = np.arange(N, dtype=np.float32)[:, None] * inv_freq[None, :]  # [N, 32]
    cos = np.cos(freqs).T.astype(np.float32)  # [32, N]
    sin = np.sin(freqs).T.astype(np.float32)
    cos64 = np.concatenate([cos, cos], axis=0)
    sinF64 = np.concatenate([-sin, sin], axis=0)
    cos2 = np.concatenate([cos64, cos64], axis=0)  # [128, N]
    sinF2 = np.concatenate([sinF64, sinF64], axis=0)
    return np.ascontiguousarray(cos2), np.ascontiguousarray(sinF2)


_PERM64 = np.concatenate([np.arange(0, D, 2), np.arange(1, D, 2)])


def _permute_heads(w):
    """Permute each head's 64 columns of w [512, 256] to [evens | odds]."""
    w = w.reshape(DIM, 4, D)[:, :, _PERM64]
    return w.reshape(DIM, 256)


def _bf16():
    import ml_dtypes
    return ml_dtypes.bfloat16


def make_in_maps(x, gamma, w_qkv, w_out):
    bf = _bf16()
    cos2, sinF2 = rope_tables()
    cos2b = cos2.astype(np.float16)
    sinF2b = sinF2.astype(np.float16)
    wg = (gamma[:, None] * w_qkv).astype(np.float32)  # fold gamma
    in_maps = []
    for c in range(NCORES):
        b, g = c // 2, c % 2
        hs = slice(g * 256, (g + 1) * 256)
        wqk_c = np.concatenate([_permute_heads(wg[:, 0:512][:, hs]),
                                _permute_heads(wg[:, 512:1024][:, hs])],
                               axis=1)
        wv_c = wg[:, 1024:1536][:, hs]
        wo_c = w_out[hs, :]
        xT = np.ascontiguousarray(x[b].T).astype(np.float32)
        in_maps.append({
            "xf": xT,
            "xb": xT.astype(bf),
            "wqk": np.ascontiguousarray(wqk_c),
            "wv": np.ascontiguousarray(wv_c).astype(bf),
            "wo": np.ascontiguousarray(wo_c).astype(bf),
            "cos2": cos2b,
            "sinF2": sinF2b,
        })
    return in_maps


_NC_CACHE = None


def _get_program():
    global _NC_CACHE
    if _NC_CACHE is None:
        _NC_CACHE = build_program()
    return _NC_CACHE


def run_cores(inputs, trace=False):
    """Run the SPMD kernel on 8 cores; returns (full_output, results)."""
    from concourse.bass_utils import run_bass_kernel_spmd

    nc = _get_program()
    in_maps = make_in_maps(inputs["x"], inputs["gamma"],
                           inputs["w_qkv"], inputs["w_out"])
    kwargs = {}
    if trace:
        _install_ntff_hook()
        kwargs = dict(trace=True, trace_cores=list(range(NCORES)))
    res = run_bass_kernel_spmd(nc, in_maps, core_ids=list(range(NCORES)),
                               **kwargs)
    out = np.empty((B, N, DIM), dtype=np.float32)
    for b in range(B):
        yTv = (res.results[2 * b]["yT"].astype(np.float32)
               + res.results[2 * b + 1]["yT"].astype(np.float32))
        out[b] = yTv.T
    return out, res


def _install_ntff_hook():
    """Register the axon NTFF profiling hook (missing antenv.axon_hooks)."""
    import sys
    import types

    if "antenv.axon_hooks" in sys.modules:
        return
    try:
        import trn_agent_boot.trn_boot as tb
        import concourse.bass_utils as bu

        mod = types.ModuleType("antenv.axon_hooks")
        hook = tb._ntff_profile_via_ctypes("/opt/axon/libaxon_pjrt.so")
        mod.get_axon_ntff_profile_hook = lambda: hook
        sys.modules["antenv.axon_hooks"] = mod
        bu.upload_artifacts = lambda tmpdir: "local://" + tmpdir
    except Exception:
        pass


def kernel(**inputs):
    out, _ = run_cores(inputs, trace=bool(os.environ.get("KERNEL_TRACE")))
    return out


# revision 9
# speedup vs baseline: 1.0925x; 1.0605x over previous
"""AttentionWithRoPE Trainium2 kernel (8-core SPMD).

Sharding: core c handles batch b = c // 2 and head-group g = c % 2
(heads 4g..4g+3).  Host sums the two partial outputs per batch.

Key ideas vs the original baseline:
- All attention matmuls are bf16 AND full 128x128-array ops.  The HAM
  clock gate only counts full-array matmuls as "PE busy": half-array
  ops (K=64 scores / M=65 attn@V) leave the PE throttled at 1.2 GHz
  for the whole attention phase (measured: cold 300us/387us).  Scores
  therefore contract over K=128 with the other head's rows zero-padded
  in the stationary operand, and attn@V pads V's 65 columns (64 dims +
  ones-denominator) to 128.  Zero padding adds no cycles.
- Q/K projection runs on RAW x in f32r (bf16-quantizing x was the
  dominant error term); the rmsnorm scale is folded into the PSUM
  drain (DVE: psum * sinv -> bf16), so projection matmuls never wait
  on the norm.
- rmsnorm: Ln+Exp (one act table set shared with the softmax Exp).
- Softmax exp is split ACT (table exp) / DVE (Schraudolph bf16:
  bits_i16 = round(A*s + B) bitcast to bf16) to share the 16.8M-elem
  exp wall across two engines.
- Output bf16 (host converts + sums the partial pairs).
"""

import os
from contextlib import ExitStack

import numpy as np

import concourse.bass as bass
import concourse.tile as tile
from concourse import bacc, mybir

B, N, DIM = 4, 2048, 512
H, D = 8, 64
ROPE_THETA = 10000.0
NCORES = 8
SCALE = D ** -0.5

F32 = mybir.dt.float32
F32R = mybir.dt.float32r
BF16 = mybir.dt.bfloat16
FP16 = mybir.dt.float16
I16 = mybir.dt.int16

# Fraction (in 8ths) of softmax-exp tiles computed on DVE via the
# Schraudolph bf16 bit trick instead of ACT.
DVE_8 = int(os.environ.get("KERNEL_DVE_8", "2"))
# Schraudolph intercept: 127*128 - C (C tunes the error balance) plus
# +0.5 if the DVE f32->i16 convert truncates instead of rounding.
DVE_B = float(os.environ.get("KERNEL_DVE_B", "16250.5"))
DVE_A = SCALE * 128.0 / np.log(2.0)  # folds the 1/sqrt(d) logit scale

LN_SQRT_DIM = float(0.5 * np.log(DIM))  # bias for sinv = exp(-0.5 ln ss + b)


def use_dve(qh, hp, kt, j):
    """Pick the engine for each softmax-exp tile.

    The first two key-tiles of every block stay on ACT so the normalize
    chain queued on DVE at the block boundary can't stall the new
    block's pipeline.  The rest spread DVE_8/8ths onto DVE.
    """
    if kt < 2:
        return False
    idx = ((qh * 2 + hp) * 16 + kt) * 2 + j
    return (idx * DVE_8) % 8 < DVE_8


def build_program():
    nc = bacc.Bacc("TRN2", target_bir_lowering=False, debug=False)

    xf_d = nc.dram_tensor("xf", [DIM, N], F32R, kind="ExternalInput").ap()
    xb_d = nc.dram_tensor("xb", [DIM, N], BF16, kind="ExternalInput").ap()
    wqk_d = nc.dram_tensor("wqk", [DIM, 512], F32R, kind="ExternalInput").ap()
    wv_d = nc.dram_tensor("wv", [DIM, 256], BF16, kind="ExternalInput").ap()
    wo_d = nc.dram_tensor("wo", [256, DIM], BF16, kind="ExternalInput").ap()
    cos_d = nc.dram_tensor("cos2", [128, N], FP16, kind="ExternalInput").ap()
    sinF_d = nc.dram_tensor("sinF2", [128, N], FP16, kind="ExternalInput").ap()
    yT = nc.dram_tensor("yT", [DIM, N], BF16, kind="ExternalOutput").ap()

    with tile.TileContext(nc) as tc:
        with ExitStack() as ctx:
            persist = ctx.enter_context(tc.tile_pool(name="persist", bufs=1))
            xsqp = ctx.enter_context(tc.tile_pool(name="xsqp", bufs=2))
            ropew = ctx.enter_context(tc.tile_pool(name="ropew", bufs=2))
            rotup = ctx.enter_context(tc.tile_pool(name="rotup", bufs=2))
            cosp = ctx.enter_context(tc.tile_pool(name="cosp", bufs=1))
            ps_s = ctx.enter_context(tc.tile_pool(name="ps_s", bufs=2, space="PSUM"))
            ps_o = ctx.enter_context(tc.tile_pool(name="ps_o", bufs=2, space="PSUM"))
            exps = ctx.enter_context(tc.tile_pool(name="exps", bufs=4))
            rcp = ctx.enter_context(tc.tile_pool(name="rcp", bufs=2))
            ysb = ctx.enter_context(tc.tile_pool(name="ysb", bufs=2))

            # ---- input DMAs ----
            xb = []
            for i in range(4):
                t = persist.tile([128, N], BF16, tag=f"xb{i}", name=f"xb{i}")
                nc.sync.dma_start(t[:], xb_d[i * 128:(i + 1) * 128, :])
                xb.append(t)
            wqk_t = []
            for i in range(4):
                t = persist.tile([128, 512], F32R, tag=f"wqk{i}", name=f"wqk{i}")
                nc.sync.dma_start(t[:], wqk_d[i * 128:(i + 1) * 128, :])
                wqk_t.append(t)
            xf = []
            for i in range(4):
                t = persist.tile([128, N], F32R, tag=f"xf{i}", name=f"xf{i}")
                nc.sync.dma_start(t[:], xf_d[i * 128:(i + 1) * 128, :])
                xf.append(t)
            cos_t = persist.tile([128, N], FP16, tag="cos", name="cos")
            nc.sync.dma_start(cos_t[:], cos_d)
            sinF_t = persist.tile([128, N], FP16, tag="sinF", name="sinF")
            nc.sync.dma_start(sinF_t[:], sinF_d)
            wv_t = []
            for i in range(4):
                t = persist.tile([128, 256], BF16, tag=f"wv{i}", name=f"wv{i}")
                nc.sync.dma_start(t[:], wv_d[i * 128:(i + 1) * 128, :])
                wv_t.append(t)
            wo_t = []
            for p in range(2):
                t = persist.tile([128, 512], BF16, tag=f"wo{p}", name=f"wo{p}")
                nc.sync.dma_start(t[:], wo_d[p * 128:(p + 1) * 128, :])
                wo_t.append(t)

            ones128 = persist.tile([128, 128], BF16, tag="ones", name="ones")
            nc.vector.memset(ones128[:], 1.0)

            # Warm-up batches: full-array matmuls that fill the PE while the
            # input DMAs land / phase A waits on DVE, so HAM reaches (and
            # keeps) K=8/8 before the projection phase.  They draw from the
            # ps_o pool (idle until attention) so they never contend with
            # the rmsnorm chunks for ps_s slots.
            def emit_warm(n, pool=None):
                warm = (pool or ps_o).tile([128, 512], F32, tag="o" if pool is None else "sc",
                                           name="warm")
                for w in range(n):
                    nc.tensor.matmul(warm[:, 0:128], ones128[:], ones128[:],
                                     start=True, stop=True)

            emit_warm(40)

            # kz[h]: zero-padded per-head roped K. Rows 0:64 = head h's 64
            # dims, rows 64:128 stay zero so the K=128 scores contraction
            # ignores the other head stacked in qr.
            # Even heads: k on rows 0:64 (zeros below); odd heads: k on rows
            # 64:128 (zeros above) — matching where that head's q lives in qr.
            kz = []
            for h in range(4):
                t = persist.tile([128, N], BF16, tag=f"kz{h}", name=f"kz{h}")
                if h % 2 == 0:
                    nc.gpsimd.memset(t[64:128, :], 0.0)
                else:
                    nc.gpsimd.memset(t[0:64, :], 0.0)
                kz.append(t)

            # v_big: 16 key-tiles x 4 heads x [64 v-dims | ones | 63 zeros]
            # (padded to 128 so attn@V is a full-array matmul).
            v_big = persist.tile([128, 16 * 512], BF16, tag="vbig", name="vbig")
            vb4 = v_big[:].rearrange("p (t h c) -> p t h c", t=16, h=4)
            nc.gpsimd.memset(v_big[:], 0.0)
            nc.vector.tensor_copy(
                vb4[:, :, :, 64],
                ones128[:, 0:64].rearrange("p (t h) -> p t h", t=16))

            # ---- phase A: rmsnorm scale (projections don't wait on it) ----
            lns = persist.tile([128, N], F32, tag="lns", name="lns")
            sinv_b = persist.tile([128, N], BF16, tag="sinvb", name="sinvb")
            bias_t = persist.tile([128, 1], F32, tag="bias", name="bias")
            nc.vector.memset(bias_t[:], LN_SQRT_DIM)

            # All Ln's land in sinv_f, then two Exp passes (bf16 copy, then
            # in-place) — grouping by table set avoids ACT table thrash.
            for c in range(4):
                cs = slice(c * 512, (c + 1) * 512)
                ss = ps_s.tile([128, 512], F32, tag="sc", name="ss")
                for i in range(4):
                    xsq = xsqp.tile([128, 512], BF16, tag="xsq", name="xsq")
                    nc.vector.tensor_mul(xsq[:], xb[i][:, cs], xb[i][:, cs])
                    nc.tensor.matmul(ss[:], ones128[:], xsq[:],
                                     start=(i == 0), stop=(i == 3))
                nc.scalar.activation(lns[:, cs], ss[:],
                                     mybir.ActivationFunctionType.Ln)
                emit_warm(8)
            # sinv = sqrt(DIM)*ss^-0.5 = exp(-0.5*ln(ss) + ln(sqrt(DIM)))
            nc.scalar.activation(sinv_b[:], lns[:],
                                 mybir.ActivationFunctionType.Exp,
                                 bias=bias_t[:], scale=-0.5)
            nc.scalar.activation(lns[:], lns[:],
                                 mybir.ActivationFunctionType.Exp,
                                 bias=bias_t[:], scale=-0.5)
            # Q/K projections run on RAW x; the rmsnorm scale rides in the
            # rope tables instead: rope(x*sinv) = raw*(cos*sinv) +
            # swap(raw)*(sinF*sinv) — sinv is per-token, rope mixes dims.
            # The whole rope intermediate path is fp16 (11-bit mantissa);
            # only the final add rounds to bf16, so the per-token scale and
            # the rotation see a single coarse rounding.
            cosS = persist.tile([128, N], FP16, tag="cosS", name="cosS")
            nc.vector.tensor_mul(cosS[:], cos_t[:], lns[:])
            sinFS = persist.tile([128, N], FP16, tag="sinFS", name="sinFS")
            nc.vector.tensor_mul(sinFS[:], sinF_t[:], lns[:])
            # xnb (bf16, for v proj), in place
            for i in range(4):
                nc.vector.tensor_mul(xb[i][:], xb[i][:], sinv_b[:])

            # ---- Q/K projection (raw x) + folded-norm RoPE ----
            # m: 0 = q heads(0,1), 1 = q heads(2,3), 2 = k(0,1), 3 = k(2,3)
            qk_dest = [persist.tile([128, N], BF16, tag=nm, name=nm)
                       for nm in ["qr0", "qr1"]]

            def emit_rope(m):
                ms = slice(m * 128, (m + 1) * 128)
                raw = ropew.tile([128, N], FP16, tag="raw", name=f"raw{m}")
                for c in range(4):
                    cs = slice(c * 512, (c + 1) * 512)
                    qk = ps_s.tile([128, 512], F32, tag="sc", name="qkps")
                    for i in range(4):
                        nc.tensor.matmul(qk[:], wqk_t[i][:, ms], xf[i][:, cs],
                                         start=(i == 0), stop=(i == 3))
                    nc.scalar.activation(raw[:, cs], qk[:],
                                         mybir.ActivationFunctionType.Copy)
                # pair-swap via SBUF->SBUF DMA (TENSOR_TENSOR needs equal
                # start partitions for SBUF operands; DMA moves partitions
                # freely), then aligned muls.
                rotu = rotup.tile([128, N], FP16, tag="rotu", name=f"rotu{m}")
                for h0 in (0, 64):
                    nc.sync.dma_start(rotu[h0:h0 + 32, :],
                                      raw[h0 + 32:h0 + 64, :])
                    nc.sync.dma_start(rotu[h0 + 32:h0 + 64, :],
                                      raw[h0:h0 + 32, :])
                nc.vector.tensor_mul(rotu[:], rotu[:], sinFS[:])
                qkcos = cosp.tile([128, N], FP16, tag="qkcos", name=f"qkcos{m}")
                nc.vector.tensor_mul(qkcos[:], raw[:], cosS[:])
                if m < 2:
                    nc.vector.tensor_add(qk_dest[m][:], qkcos[:], rotu[:])
                else:
                    # K: write each head's rows straight into its
                    # zero-padded kz tile (partition-aligned adds).
                    hp = m - 2
                    nc.vector.tensor_add(kz[2 * hp][0:64, :],
                                         qkcos[0:64, :], rotu[0:64, :])
                    nc.vector.tensor_add(kz[2 * hp + 1][64:128, :],
                                         qkcos[64:128, :], rotu[64:128, :])

            def emit_v():
                for tt in range(16):
                    ts = slice(tt * 128, (tt + 1) * 128)
                    vp = ps_s.tile([128, 512], F32, tag="sc", name="vps")
                    for i in range(4):
                        nc.tensor.matmul(vp[:, 0:256], xb[i][:, ts], wv_t[i][:],
                                         start=(i == 0), stop=(i == 3))
                    nc.scalar.activation(
                        vb4[:, tt, :, 0:64],
                        vp[:, 0:256].rearrange("p (h c) -> p h c", h=4),
                        mybir.ActivationFunctionType.Copy)

            # ---- attention ----
            outd = [[None, None], [None, None]]

            def emit_attention(qh, hp):
                qr = qk_dest[hp]
                o_ps = [ps_o.tile([128, 1024], F32, tag="o", name="o")
                        for _ in range(2)]
                def emit_av(kt, e_t):
                    for j in range(2):
                        h = 2 * hp + j
                        vcol = slice(512 * kt + 128 * h,
                                     512 * kt + 128 * h + 128)
                        for sub in range(2):
                            ss_ = slice(sub * 512, (sub + 1) * 512)
                            nc.tensor.matmul(
                                o_ps[j][:, ss_],
                                v_big[:, vcol], e_t[j][:, ss_],
                                start=(kt == 0), stop=(kt == 15),
                                skip_group_check=True)

                # software pipeline: attn@V for key-tile kt-1 is emitted
                # after scores(kt), so the PE has work while exp(kt) runs.
                prev = None
                for kt in range(16):
                    ks = slice(kt * 128, (kt + 1) * 128)
                    e_t = []
                    for j in range(2):
                        s_ps = ps_s.tile([128, 1024], F32, tag="sc", name="sc")
                        for sub in range(2):
                            qs = slice(qh * 1024 + sub * 512,
                                       qh * 1024 + (sub + 1) * 512)
                            nc.tensor.matmul(
                                s_ps[:, sub * 512:(sub + 1) * 512],
                                kz[2 * hp + j][:, ks], qr[:, qs],
                                start=True, stop=True)
                        e = exps.tile([128, 1024], BF16, tag="e", name="e")
                        if use_dve(qh, hp, kt, j):
                            nc.vector.tensor_scalar(
                                e[:].bitcast(I16), s_ps[:],
                                float(DVE_A), float(DVE_B),
                                mybir.AluOpType.mult, mybir.AluOpType.add)
                        else:
                            nc.scalar.activation(
                                e[:], s_ps[:],
                                mybir.ActivationFunctionType.Exp, scale=SCALE)
                        e_t.append(e)
                    if prev is not None:
                        emit_av(kt - 1, prev)
                    prev = e_t
                emit_av(15, prev)
                od = persist.tile([128, 1024], BF16, tag=f"od{hp}{qh}",
                                  name=f"od{hp}{qh}")
                outd[hp][qh] = od
                for j in range(2):
                    js = slice(j * 64, (j + 1) * 64)
                    rrow = rcp.tile([1, 1024], F32, tag="rrow", name="rrow")
                    dcopy = rcp.tile([1, 1024], F32, tag="dcopy", name="dcopy")
                    nc.vector.tensor_copy(dcopy[:], o_ps[j][64:65, :])
                    nc.vector.reciprocal_approx_fast(rrow[:], dcopy[:])
                    rfull = rcp.tile([64, 1024], F32, tag="rfull", name="rfull")
                    nc.gpsimd.partition_broadcast(rfull[:], rrow[:])
                    nc.vector.tensor_mul(od[js, :], o_ps[j][0:64, :], rfull[:])

            def emit_proj(qh):
                for om in range(4):
                    oms = slice(om * 128, (om + 1) * 128)
                    yp = ps_s.tile([128, 1024], F32, tag="sc", name="yp")
                    for sub in range(2):
                        ss_ = slice(sub * 512, (sub + 1) * 512)
                        for p in range(2):
                            nc.tensor.matmul(
                                yp[:, ss_], wo_t[p][:, oms],
                                outd[p][qh][:, ss_],
                                start=(p == 0), stop=(p == 1))
                    yo = ysb.tile([128, 1024], BF16, tag="y", name="y")
                    if om % 2 == 0:
                        nc.scalar.activation(yo[:], yp[:],
                                             mybir.ActivationFunctionType.Copy)
                    else:
                        nc.vector.tensor_copy(yo[:], yp[:])
                    nc.sync.dma_start(
                        yT[oms, qh * 1024:(qh + 1) * 1024], yo[:])

            # ---- emission order: the (0,0) block's K and Q first (they
            # only need raw x), V before attention, second pair last ----
            emit_rope(2)
            emit_rope(0)
            emit_v()
            emit_rope(3)
            emit_rope(1)
            # proj(0) goes after att(1,1): its matmuls run (and keep HAM
            # warm) while norm(1,1) drains, instead of stalling the PE
            # twice waiting for each qh's last normalize.
            emit_attention(0, 0)
            emit_attention(0, 1)
            emit_attention(1, 0)
            emit_attention(1, 1)
            # keep the PE fed/warm while norm(1,1) drains (ps_s: the ps_o
            # slots are exactly what norm is still holding)
            emit_warm(24, pool=ps_s)
            emit_proj(0)
            emit_proj(1)

    nc.compile()
    return nc


def rope_tables():
    """cos / sign-folded sin tables, [evens | odds] row order, two 64-row
    head slots stacked twice to fill 128 partitions."""
    inv_freq = (1.0 / (ROPE_THETA ** (np.arange(0, D, 2, dtype=np.float32) / D)))
    freqs = np.arange(N, dtype=np.float32)[:, None] * inv_freq[None, :]  # [N, 32]
    cos = np.cos(freqs).T.astype(np.float32)  # [32, N]
    sin = np.sin(freqs).T.astype(np.float32)
    cos64 = np.concatenate([cos, cos], axis=0)
    sinF64 = np.concatenate([-sin, sin], axis=0)
    cos2 = np.concatenate([cos64, cos64], axis=0)  # [128, N]
    sinF2 = np.concatenate([sinF64, sinF64], axis=0)
    return np.ascontiguousarray(cos2), np.ascontiguousarray(sinF2)


_PERM64 = np.concatenate([np.arange(0, D, 2), np.arange(1, D, 2)])


def _permute_heads(w):
    """Permute each head's 64 columns of w [512, 256] to [evens | odds]."""
    w = w.reshape(DIM, 4, D)[:, :, _PERM64]
    return w.reshape(DIM, 256)


def _bf16():
    import ml_dtypes
    return ml_dtypes.bfloat16


def make_in_maps(x, gamma, w_qkv, w_out):
    bf = _bf16()
    cos2, sinF2 = rope_tables()
    cos2b = cos2.astype(np.float16)
    sinF2b = sinF2.astype(np.float16)
    wg = (gamma[:, None] * w_qkv).astype(np.float32)  # fold gamma
    in_maps = []
    for c in range(NCORES):
        b, g = c // 2, c % 2
        hs = slice(g * 256, (g + 1) * 256)
        wqk_c = np.concatenate([_permute_heads(wg[:, 0:512][:, hs]),
                                _permute_heads(wg[:, 512:1024][:, hs])],
                               axis=1)
        wv_c = wg[:, 1024:1536][:, hs]
        wo_c = w_out[hs, :]
        xT = np.ascontiguousarray(x[b].T).astype(np.float32)
        in_maps.append({
            "xf": xT,
            "xb": xT.astype(bf),
            "wqk": np.ascontiguousarray(wqk_c),
            "wv": np.ascontiguousarray(wv_c).astype(bf),
            "wo": np.ascontiguousarray(wo_c).astype(bf),
            "cos2": cos2b,
            "sinF2": sinF2b,
        })
    return in_maps


_NC_CACHE = None


def _get_program():
    global _NC_CACHE
    if _NC_CACHE is None:
        _NC_CACHE = build_program()
    return _NC_CACHE


def run_cores(inputs, trace=False):
    """Run the SPMD kernel on 8 cores; returns (full_output, results)."""
    from concourse.bass_utils import run_bass_kernel_spmd

    nc = _get_program()
    in_maps = make_in_maps(inputs["x"], inputs["gamma"],
                           inputs["w_qkv"], inputs["w_out"])
    kwargs = {}
    if trace:
        _install_ntff_hook()
        kwargs = dict(trace=True, trace_cores=list(range(NCORES)))
    res = run_bass_kernel_spmd(nc, in_maps, core_ids=list(range(NCORES)),
                               **kwargs)
    out = np.empty((B, N, DIM), dtype=np.float32)
    for b in range(B):
        yTv = (res.results[2 * b]["yT"].astype(np.float32)
               + res.results[2 * b + 1]["yT"].astype(np.float32))
        out[b] = yTv.T
    return out, res


def _install_ntff_hook():
    """Register the axon NTFF profiling hook (missing antenv.axon_hooks)."""
    import sys
    import types

    if "antenv.axon_hooks" in sys.modules:
        return
    try:
        import trn_agent_boot.trn_boot as tb
        import concourse.bass_utils as bu

        mod = types.ModuleType("antenv.axon_hooks")
        hook = tb._ntff_profile_via_ctypes("/opt/axon/libaxon_pjrt.so")
        mod.get_axon_ntff_profile_hook = lambda: hook
        sys.modules["antenv.axon_hooks"] = mod
        bu.upload_artifacts = lambda tmpdir: "local://" + tmpdir
    except Exception:
        pass


def kernel(**inputs):
    out, _ = run_cores(inputs, trace=bool(os.environ.get("KERNEL_TRACE")))
    return out
